# revision 22
# baseline (speedup 1.0000x reference)
"""DualAttention2d Trainium2 kernel.

Sharding: 8 cores = 4 samples x {spatial-attention branch, channel-attention
branch}. Core c < 4 computes the spatial branch of sample c; core c >= 4
computes the channel branch of sample c-4. Host sums the two branch outputs.

Single SPMD program; branch divergence via tc.If(partition_id < 4).

Layout notes:
- Feature maps on-chip as [4 blocks][128 chan, S] with S = 64*64 = 4096.
- Conv inputs live in a zero-padded [128, 66*66] buffer (1-px halo); a 3x3
  conv is 9 shifted matmuls accumulated in PSUM over 4 channel blocks.
- BN is folded into conv weights/bias on the host. alpha is folded into the
  v-projection, beta into the channel-attention softmax normalization.
- Matmuls run in float32r (full PE rate at N>=256, ~1e-4 rel precision).
  Attention probabilities are bf16; they are transposed for the o-matmul by
  PE transposes (128x128 tiles) evicted via ScalarE into two half-buffers.
- Conv1 is fused with the q/k/vT projections (st-pair outer loop, evict
  tiles consumed in SBUF); conv weights are SBUF-resident per output block.
- DMAs are batched (2-4 tiles per transfer) and split between the SP (HWDGE)
  and GpSimd (SWDGE) queues to keep issue cost off the critical path.
- The attention middle is software-pipelined over (group, blk) steps:
  logits matmuls run two steps ahead and softmax one step ahead of the PE
  transposes, so the in-order PE queue never waits on Act/DVE softmax
  latency. Transposes land 4-to-a-PSUM-tile and evict in one strided
  activation; the o-matmul accumulates all 4 channel blocks in a single
  pass (4 PSUM banks), reading each vT chunk once per group. The spatial
  branch skips c1t production entirely (only the channel branch reads it).
- The per-blk negmax is computed in stage L (right after the st loop, two
  steps before its consumer) so exp never waits at the tail of the DVE
  queue behind the lookahead logits copies.
- Cost model (TimelineSim): spatial branch ~1.19 ms, channel ~0.73 ms per
  core (vs ~1.38/0.73 ms before pipelining, ~1.75/0.73 ms phase-serial).
"""

import numpy as np

import concourse.bacc as bacc
import concourse.mybir as mybir
import concourse.tile as tile
from concourse.bass_utils import run_bass_kernel_spmd

B, C, H, W = 4, 512, 64, 64
S = H * W            # 4096
CI = 64              # q/k channels
P = 128
NB = C // P          # 4 channel blocks
PW = 66              # padded row width
PR = 66              # padded rows (1 zero row top/bottom)
PAD = PW * PR        # 4356
NST = S // 512       # 8 s-tiles of 512
NCH = S // P         # 32 s-chunks of 128
EPS = 1e-5

F32 = mybir.dt.float32
F32R = mybir.dt.float32r
BF16 = mybir.dt.bfloat16
AF = mybir.ActivationFunctionType
AX = mybir.AxisListType

_CACHE = {}


def _pad_view(xpad_ap, st, dy=1, dx=1):
    """View of padded buffer [128, PAD] covering s-tile `st` (8 image rows x 64
    cols) shifted by tap (dy, dx) in {0,1,2}^2. dy=dx=1 is the centered view."""
    v = xpad_ap.rearrange("p (r w) -> p r w", w=PW)
    r0 = st * 8 + dy
    return v[:, r0:r0 + 8, dx:dx + 64]


def build(branch=None):
    """branch=None: SPMD program with If/Else on partition id.
    branch="spatial"/"channel": single-branch program (analysis/timing)."""
    nc = bacc.Bacc("TRN2", target_bir_lowering=False, debug=False,
                   num_devices=8)

    # ---- I/O ----
    x_d = nc.dram_tensor("xpad", [NB, P, PAD], F32R, kind="ExternalInput")
    # conv weights pre-arranged host-side: [ob, tap, cb, ci, o]
    w1_d = nc.dram_tensor("w1", [NB, 36, P, P], F32R, kind="ExternalInput")
    b1_d = nc.dram_tensor("b1", [NB, P, 1], F32, kind="ExternalInput")
    w2_d = nc.dram_tensor("w2", [NB, 36, P, P], F32R, kind="ExternalInput")
    b2_d = nc.dram_tensor("b2", [NB, P, 1], F32, kind="ExternalInput")
    qw_d = nc.dram_tensor("qw", [NB, P, CI], F32R, kind="ExternalInput")
    kw_d = nc.dram_tensor("kw", [NB, P, CI], F32R, kind="ExternalInput")
    vw_d = nc.dram_tensor("vw", [NB, P, 512], F32R, kind="ExternalInput")
    qb_d = nc.dram_tensor("qb", [CI, 1], F32, kind="ExternalInput")
    kb_d = nc.dram_tensor("kb", [CI, 1], F32, kind="ExternalInput")
    vba_d = nc.dram_tensor("vba", [NB, P, 1], F32, kind="ExternalInput")
    beta_d = nc.dram_tensor("betat", [P, 1], F32, kind="ExternalInput")
    idr_d = nc.dram_tensor("identr", [P, P], F32R, kind="ExternalInput")
    idb_d = nc.dram_tensor("identb", [P, P], BF16, kind="ExternalInput")
    out_d = nc.dram_tensor("out", [NB, P, S], F32, kind="ExternalOutput")

    # ---- internal DRAM scratch ----
    s1_d = nc.dram_tensor("s1f", [NB, P, S], F32R, kind="Internal")
    c1t_d = nc.dram_tensor("c1t", [NCH, P, 512], F32R, kind="Internal")
    q_d = nc.dram_tensor("qs", [CI, S], F32R, kind="Internal")
    k_d = nc.dram_tensor("ks", [CI, S], F32R, kind="Internal")
    vt_d = nc.dram_tensor("vts", [NCH, P, 512], BF16, kind="Internal")

    with tile.TileContext(nc) as tc:
        from contextlib import ExitStack

        # ---- global pools (whole kernel) ----
        gctx = ExitStack()
        psA = gctx.enter_context(tc.tile_pool(name="psA", bufs=6,
                                              space="PSUM"))
        psT = gctx.enter_context(tc.tile_pool(name="psT", bufs=2,
                                              space="PSUM"))
        xpadp = gctx.enter_context(tc.tile_pool(name="xpadp", bufs=NB))
        consts = gctx.enter_context(tc.tile_pool(name="consts", bufs=1))
        b512 = gctx.enter_context(tc.tile_pool(name="b512", bufs=3))
        statp = gctx.enter_context(tc.tile_pool(name="statp", bufs=12))

        # ---- constants ----
        ident_r = consts.tile([P, P], F32R, name="ident_r")
        nc.sync.dma_start(ident_r[:], idr_d.ap())
        ident_b = consts.tile([P, P], BF16, name="ident_b")
        nc.sync.dma_start(ident_b[:], idb_d.ap())
        qw_t = [consts.tile([P, CI], F32R, name=f"qw{i}") for i in range(NB)]
        kw_t = [consts.tile([P, CI], F32R, name=f"kw{i}") for i in range(NB)]
        vw_t = [consts.tile([P, 512], F32R, name=f"vw{i}") for i in range(NB)]
        b1_t = [consts.tile([P, 1], F32, name=f"b1{i}") for i in range(NB)]
        b2_t = [consts.tile([P, 1], F32, name=f"b2{i}") for i in range(NB)]
        vba_t = [consts.tile([P, 1], F32, name=f"vba{i}") for i in range(NB)]
        qb_t = consts.tile([CI, 1], F32, name="qbt")
        kb_t = consts.tile([CI, 1], F32, name="kbt")
        beta_t = consts.tile([P, 1], F32, name="betat_sb")
        for i in range(NB):
            nc.sync.dma_start(qw_t[i][:], qw_d[i])
            nc.sync.dma_start(kw_t[i][:], kw_d[i])
            nc.sync.dma_start(vw_t[i][:], vw_d[i])
            nc.sync.dma_start(b1_t[i][:], b1_d[i])
            nc.sync.dma_start(b2_t[i][:], b2_d[i])
            nc.sync.dma_start(vba_t[i][:], vba_d[i])
        nc.sync.dma_start(qb_t[:], qb_d.ap())
        nc.sync.dma_start(kb_t[:], kb_d.ap())
        nc.sync.dma_start(beta_t[:], beta_d.ap())

        # ---- load padded input ----
        xpad = [xpadp.tile([P, PAD], F32R, tag="xp", name=f"xpad{i}")
                for i in range(NB)]
        for i in range(NB):
            hh = PAD // 2
            nc.sync.dma_start(xpad[i][:, :hh], x_d[i, :, :hh])
            nc.gpsimd.dma_start(xpad[i][:, hh:], x_d[i, :, hh:])

        def load_wres(wpool, w_dram, ob):
            """The 36 [128,128] stationaries of one conv output block."""
            wres = wpool.tile([P, 36 * P], F32R, tag="wres", name="wres")
            nc.sync.dma_start(
                wres[:].rearrange("p (k o) -> p k o", o=P),
                w_dram[ob].rearrange("k p o -> p k o"))
            return wres

        def conv1_pair(wres, ob, st0, bounce, b1ref):
            """One conv over s-tiles (st0, st0+1) for output block ob; returns
            the evicted [128,1024] relu tile; also writes s1_d and c1t_d."""
            ps = [psA.tile([P, 512], F32, tag="mm", name=f"c1p{sl}")
                  for sl in range(2)]
            for tci in range(36):
                cb, tap = tci // 9, tci % 9
                dy, dx = tap // 3, tap % 3
                for sl in range(2):
                    nc.tensor.matmul(
                        ps[sl][:], wres[:, tci * P:(tci + 1) * P],
                        _pad_view(xpad[cb][:], st0 + sl, dy, dx),
                        start=(tci == 0), stop=(tci == 35))
            sb = bounce.tile([P, 1024], F32R, tag="bn", name=f"sb{ob}")
            for sl in range(2):
                nc.scalar.activation(sb[:, sl * 512:(sl + 1) * 512],
                                     ps[sl][:], AF.Relu, bias=b1ref[ob][:])
            nc.gpsimd.dma_start(
                s1_d[ob, :, st0 * 512:(st0 + 2) * 512], sb[:])
            return sb

        def c1t_out(sb, ob, st0, tb4):
            """Transpose the pair-tile into c1t_d chunks (8 chunks)."""
            for sl in range(2):
                tb = tb4.tile([P, 512], F32R, tag="t4", name="tb")
                for j in range(4):
                    pt = psT.tile([P, P], F32R, tag="tp", name="pt")
                    nc.tensor.transpose(
                        pt[:], sb[:, sl * 512 + j * P:sl * 512 + (j + 1) * P],
                        ident_r[:])
                    nc.scalar.activation(tb[:, j * P:(j + 1) * P], pt[:],
                                         AF.Identity)
                st = st0 + sl
                nc.gpsimd.dma_start(
                    c1t_d.ap()[st * 4:st * 4 + 4, :, ob * P:(ob + 1) * P]
                    .rearrange("j p c -> p j c"),
                    tb[:].rearrange("p (j c) -> p j c", c=P))

        def spatial_middle():
            # long-lived attention inputs
            resctx = ExitStack()
            kqp = resctx.enter_context(tc.tile_pool(name="kqp", bufs=1))
            kg = kqp.tile([CI, S], F32R, tag="kg", name="kg")

            # ---- conv1 fused with q/k/vT production, st-pair outer ----
            # (no c1t_out here: c1t is only read by the channel branch)
            with ExitStack() as c1ctx:
                wp = c1ctx.enter_context(tc.tile_pool(name="wp1", bufs=2))
                bounce = c1ctx.enter_context(tc.tile_pool(name="bn1", bufs=5))
                vtbp = c1ctx.enter_context(tc.tile_pool(name="vtbp", bufs=2))
                for pair in range(NST // 2):
                    st0 = pair * 2
                    sbs = []
                    for ob in range(NB):
                        wres = load_wres(wp, w1_d.ap(), ob)
                        sb = conv1_pair(wres, ob, st0, bounce, b1_t)
                        sbs.append(sb)
                    # q, k, vT for the two s-tiles of this pair
                    for sl in range(2):
                        st = st0 + sl
                        ssl = slice(sl * 512, (sl + 1) * 512)
                        pq = psA.tile([CI, 512], F32, tag="mm", name="pq")
                        pk = psA.tile([CI, 512], F32, tag="mm", name="pk")
                        for cb in range(NB):
                            nc.tensor.matmul(pq[:], qw_t[cb][:],
                                             sbs[cb][:, ssl],
                                             start=(cb == 0),
                                             stop=(cb == NB - 1))
                        for cb in range(NB):
                            nc.tensor.matmul(pk[:], kw_t[cb][:],
                                             sbs[cb][:, ssl],
                                             start=(cb == 0),
                                             stop=(cb == NB - 1))
                        qsb = b512.tile([CI, 512], F32R, tag="bn",
                                        name="qsb")
                        nc.scalar.activation(qsb[:], pq[:], AF.Identity,
                                             bias=qb_t[:])
                        nc.gpsimd.dma_start(
                            q_d.ap()[:, st * 512:(st + 1) * 512], qsb[:])
                        nc.scalar.activation(kg[:, st * 512:(st + 1) * 512],
                                             pk[:], AF.Identity, bias=kb_t[:])
                        vtb = vtbp.tile([P, 2048], BF16, tag="vtb",
                                        name=f"vtb{st}")
                        for j in range(4):
                            pv = psA.tile([P, 512], F32, tag="mm", name="pv")
                            for cb in range(NB):
                                nc.tensor.matmul(
                                    pv[:],
                                    sbs[cb][:, sl * 512 + j * P:
                                            sl * 512 + (j + 1) * P],
                                    vw_t[cb][:], start=(cb == 0),
                                    stop=(cb == NB - 1))
                            nc.scalar.activation(
                                vtb[:, j * 512:(j + 1) * 512], pv[:],
                                AF.Identity)
                        nc.gpsimd.dma_start(
                            vt_d.ap()[st * 4:st * 4 + 4].rearrange(
                                "j p n -> p j n"),
                            vtb[:].rearrange("p (j n) -> p j n", n=512))

            # ---- attention, software-pipelined over (g, blk) steps ----
            # Stage L (logits matmuls + copies + maxes) runs two steps ahead
            # and stage SM (softmax: negmax/exp/recip/mul) one step ahead of
            # stage TR (PE transposes), so the in-order PE queue never waits
            # on Act/DVE softmax latency. The o-matmul accumulates all four
            # channel blocks in one pass (4 PSUM banks), reading each vT
            # chunk once per group.
            with ExitStack() as attctx:
                qgp = attctx.enter_context(tc.tile_pool(name="qgp", bufs=2))
                logp = attctx.enter_context(tc.tile_pool(name="logp", bufs=2))
                probp = attctx.enter_context(tc.tile_pool(name="probp",
                                                          bufs=2))
                attTpA = attctx.enter_context(tc.tile_pool(name="attTpA",
                                                           bufs=1))
                attTpB = attctx.enter_context(tc.tile_pool(name="attTpB",
                                                           bufs=1))
                vtip = attctx.enter_context(tc.tile_pool(name="vtip", bufs=3))
                s1rp = attctx.enter_context(tc.tile_pool(name="s1rp", bufs=1))
                NSTEP = NST * 4
                qgs, logst, smst, atts = {}, {}, {}, {}

                def issue_L(i):
                    g, blk = divmod(i, 4)
                    if blk == 0:
                        qg = qgp.tile([CI, 512], F32R, tag="qg", name="qg")
                        nc.sync.dma_start(
                            qg[:], q_d.ap()[:, g * 512:(g + 1) * 512])
                        qgs[g] = qg
                    qg = qgs[g]
                    logits = logp.tile([P, S], F32, tag="lg", name="logits")
                    pmax = statp.tile([P, 8], F32, tag="pm", name="pmax")
                    for st in range(NST):
                        pl = psA.tile([P, 512], F32, tag="mm", name="pl")
                        nc.tensor.matmul(
                            pl[:], qg[:, blk * P:(blk + 1) * P],
                            kg[:, st * 512:(st + 1) * 512],
                            start=True, stop=True)
                        nc.vector.tensor_copy(
                            logits[:, st * 512:(st + 1) * 512], pl[:])
                        nc.vector.reduce_max(pmax[:, st:st + 1], pl[:],
                                             axis=AX.X)
                    # negmax here (stage L, two steps early) rather than in
                    # SM: it only needs pmax, and hoisting it off the tail
                    # of the DVE queue gives exp a full step of slack
                    negmax = statp.tile([P, 1], F32, tag="st", name="negmax")
                    nc.vector.reduce_max(negmax[:], pmax[:], axis=AX.X,
                                         negate=True)
                    logst[i] = (logits, negmax)

                def issue_SM(i):
                    logits, negmax = logst.pop(i)
                    probs = probp.tile([P, S], BF16, tag="pb", name="probs")
                    rowsum = statp.tile([P, 1], F32, tag="st", name="rowsum")
                    nc.scalar.activation(probs[:], logits[:], AF.Exp,
                                         bias=negmax[:], accum_out=rowsum[:])
                    recip = statp.tile([P, 1], F32, tag="st", name="recip")
                    nc.vector.reciprocal(recip[:], rowsum[:])
                    nc.vector.tensor_scalar_mul(probs[:], probs[:], recip[:])
                    smst[i] = probs

                def issue_TR(i):
                    g, blk = divmod(i, 4)
                    if blk == 0:
                        attA = attTpA.tile([P, NCH * 256], BF16, tag="attA",
                                           name=f"attA{g}")
                        attB = attTpB.tile([P, NCH * 256], BF16, tag="attB",
                                           name=f"attB{g}")
                        atts[g] = (attA, attB)
                    attA, attB = atts[g]
                    probs = smst.pop(i)
                    for q4 in range(NCH // 4):
                        pt = psT.tile([P, 512], BF16, tag="tp", name="pt")
                        for k in range(4):
                            j = q4 * 4 + k
                            nc.tensor.transpose(
                                pt[:, k * P:(k + 1) * P],
                                probs[:, j * P:(j + 1) * P], ident_b[:])
                        ah = attA if q4 < NCH // 8 else attB
                        jh0 = (q4 * 4) % (NCH // 2)
                        ahv = ah[:].rearrange("p (j b c) -> p j b c",
                                              b=4, c=P)
                        nc.scalar.activation(
                            ahv[:, jh0:jh0 + 4, blk],
                            pt[:].rearrange("p (k c) -> p k c", c=P),
                            AF.Identity)

                def issue_PO(g):
                    attA, attB = atts.pop(g)
                    po = [psA.tile([P, 512], F32, tag="mm", name=f"po{i}")
                          for i in range(NB)]
                    for j4 in range(NCH // 4):
                        vt = vtip.tile([P, 4 * 512], BF16, tag="vti",
                                       name="vt")
                        nc.sync.dma_start(
                            vt[:].rearrange("p (j n) -> p j n", n=512),
                            vt_d.ap()[j4 * 4:j4 * 4 + 4].rearrange(
                                "j p n -> p j n"))
                        for jj in range(4):
                            j = j4 * 4 + jj
                            ahalf = attA if j < NCH // 2 else attB
                            jh = j % (NCH // 2)
                            for cb in range(NB):
                                nc.tensor.matmul(
                                    po[cb][:],
                                    vt[:, jj * 512 + cb * P:
                                       jj * 512 + (cb + 1) * P],
                                    ahalf[:, jh * 512:(jh + 1) * 512],
                                    start=(j == 0), stop=(j == NCH - 1))
                    s1r = s1rp.tile([P, NB * 512], F32R, tag="s1r",
                                    name="s1r")
                    nc.sync.dma_start(
                        s1r[:].rearrange("p (b n) -> p b n", n=512),
                        s1_d.ap()[:, :, g * 512:(g + 1) * 512].rearrange(
                            "b p n -> p b n"))
                    for cb in range(NB):
                        ob_sb = b512.tile([P, 512], F32, tag="bn",
                                          name="obsb")
                        nc.scalar.activation(ob_sb[:], po[cb][:],
                                             AF.Identity, bias=vba_t[cb][:])
                        nc.vector.tensor_add(
                            _pad_view(xpad[cb][:], g), ob_sb[:],
                            s1r[:, cb * 512:(cb + 1) * 512])

                issue_L(0)
                issue_L(1)
                issue_SM(0)
                for i in range(NSTEP):
                    if i + 2 < NSTEP:
                        issue_L(i + 2)
                    if i + 1 < NSTEP:
                        issue_SM(i + 1)
                    issue_TR(i)
                    if i % 4 == 3:
                        issue_PO(i // 4)
            resctx.close()
            conv2()

        def channel_middle():
            # ---- conv1 (st-pair outer) + c1T production ----
            with ExitStack() as c1ctx:
                wp = c1ctx.enter_context(tc.tile_pool(name="wp1c", bufs=2))
                bounce = c1ctx.enter_context(tc.tile_pool(name="bn1c",
                                                          bufs=3))
                tb4 = c1ctx.enter_context(tc.tile_pool(name="tb41c", bufs=2))
                for pair in range(NST // 2):
                    st0 = pair * 2
                    for ob in range(NB):
                        wres = load_wres(wp, w1_d.ap(), ob)
                        sb = conv1_pair(wres, ob, st0, bounce, b1_t)
                        c1t_out(sb, ob, st0, tb4)

            with ExitStack() as chctx:
                c1tp = chctx.enter_context(tc.tile_pool(name="c1tp", bufs=2))
                cattp = chctx.enter_context(tc.tile_pool(name="cattp",
                                                         bufs=NB))
                # G = c1 @ c1^T via transposed chunks
                pg = [psA.tile([P, 512], F32, tag="mm", name=f"pg{cb}")
                      for cb in range(NB)]
                for j2 in range(NCH // 2):
                    c1t = c1tp.tile([P, 1024], F32R, tag="c1t", name="c1tin")
                    nc.sync.dma_start(
                        c1t[:].rearrange("p (j n) -> p j n", n=512),
                        c1t_d.ap()[j2 * 2:j2 * 2 + 2].rearrange(
                            "j p n -> p j n"))
                    for jj in range(2):
                        j = j2 * 2 + jj
                        ch = c1t[:, jj * 512:(jj + 1) * 512]
                        for cb in range(NB):
                            nc.tensor.matmul(pg[cb][:],
                                             ch[:, cb * P:(cb + 1) * P],
                                             ch[:], start=(j == 0),
                                             stop=(j == NCH - 1))
                catt = []
                for cb in range(NB):
                    negmax = statp.tile([P, 1], F32, tag="st", name="negmax")
                    nc.vector.reduce_max(negmax[:], pg[cb][:], axis=AX.X,
                                         negate=True)
                    ct = cattp.tile([P, 512], F32R, tag="ct",
                                    name=f"catt{cb}")
                    rowsum = statp.tile([P, 1], F32, tag="st", name="rowsum")
                    nc.scalar.activation(ct[:], pg[cb][:], AF.Exp,
                                         bias=negmax[:], accum_out=rowsum[:])
                    recip = statp.tile([P, 1], F32, tag="st", name="recip")
                    nc.vector.reciprocal(recip[:], rowsum[:])
                    # fold beta in: catt = beta * softmax(G)
                    nc.vector.tensor_mul(recip[:], recip[:], beta_t[:])
                    nc.scalar.activation(ct[:], ct[:], AF.Identity,
                                         scale=recip[:])
                    catt.append(ct)
                for st in range(NST):
                    c1s = c1tp.tile([P, NB, 512], F32R, tag="c4", name="c1s")
                    nc.sync.dma_start(
                        c1s[:],
                        s1_d.ap()[:, :, st * 512:(st + 1) * 512].rearrange(
                            "b p n -> p b n"))
                    for kb in range(NB):
                        pc = psA.tile([P, 512], F32, tag="mm", name="pc")
                        for cb in range(NB):
                            nc.tensor.matmul(
                                pc[:], catt[cb][:, kb * P:(kb + 1) * P],
                                c1s[:, cb], start=(cb == 0),
                                stop=(cb == NB - 1))
                        nc.vector.tensor_add(
                            _pad_view(xpad[kb][:], st), pc[:], c1s[:, kb])
            conv2()

        def conv2():
            # st-outer so it can chase the middle's residual writes
            with ExitStack() as c2ctx:
                wp = c2ctx.enter_context(tc.tile_pool(name="wp2", bufs=2))
                bounce2 = c2ctx.enter_context(tc.tile_pool(name="bn2",
                                                           bufs=2))
                for pair in range(NST // 2):
                    st0 = pair * 2
                    for ob in range(NB):
                        wres = load_wres(wp, w2_d.ap(), ob)
                        ps = [psA.tile([P, 512], F32, tag="mm",
                                       name=f"c2p{sl}") for sl in range(2)]
                        for tci in range(36):
                            cb, tap = tci // 9, tci % 9
                            dy, dx = tap // 3, tap % 3
                            for sl in range(2):
                                nc.tensor.matmul(
                                    ps[sl][:], wres[:, tci * P:(tci + 1) * P],
                                    _pad_view(xpad[cb][:], st0 + sl, dy, dx),
                                    start=(tci == 0), stop=(tci == 35))
                        sb = bounce2.tile([P, 1024], F32, tag="bn",
                                          name=f"ob{ob}")
                        for sl in range(2):
                            nc.scalar.activation(
                                sb[:, sl * 512:(sl + 1) * 512], ps[sl][:],
                                AF.Relu, bias=b2_t[ob][:])
                        nc.gpsimd.dma_start(
                            out_d[ob, :, st0 * 512:(st0 + 2) * 512], sb[:])

        if branch == "spatial":
            spatial_middle()
        elif branch == "channel":
            channel_middle()
        else:
            pid = nc.partition_id()
            with tc.If(pid < 4) as cmp:
                spatial_middle()
            with cmp.Else():
                channel_middle()

        gctx.close()

    nc.compile()
    return nc


def _fold_conv(w, g, b, m, v):
    scale = g / np.sqrt(v + EPS)
    wf = (w * scale[:, None, None, None]).astype(np.float32)
    bf = (b - m * scale).astype(np.float32)
    # [O, CI, 3, 3] -> [ob, (cb tap), ci, o]
    wt = wf.transpose(2, 3, 1, 0).reshape(9, NB, P, NB, P).transpose(
        3, 1, 0, 2, 4).reshape(NB, 36, P, P)
    return np.ascontiguousarray(wt), bf.reshape(NB, P, 1)


def _pad_x(x):
    # x: [C, H, W] -> [NB, P, PAD]
    xp = np.zeros((NB, P, PR, PW), np.float32)
    xp[:, :, 1:65, 1:65] = x.reshape(NB, P, H, W)
    return xp.reshape(NB, P, PAD)


def prep_inputs(inputs):
    """Build the 8 per-core input maps from the full problem inputs."""
    x = np.asarray(inputs["x"], np.float32)
    alpha = float(np.asarray(inputs["alpha"]).reshape(-1)[0])
    beta = float(np.asarray(inputs["beta"]).reshape(-1)[0])

    w1s, b1s = _fold_conv(np.asarray(inputs["sa_w1"]), inputs["sa_g1"],
                          inputs["sa_b1"], inputs["sa_m1"], inputs["sa_v1"])
    w2s, b2s = _fold_conv(np.asarray(inputs["sa_w2"]), inputs["sa_g2"],
                          inputs["sa_b2"], inputs["sa_m2"], inputs["sa_v2"])
    w1c, b1c = _fold_conv(np.asarray(inputs["ca_w1"]), inputs["ca_g1"],
                          inputs["ca_b1"], inputs["ca_m1"], inputs["ca_v1"])
    w2c, b2c = _fold_conv(np.asarray(inputs["ca_w2"]), inputs["ca_g2"],
                          inputs["ca_b2"], inputs["ca_m2"], inputs["ca_v2"])

    qw = np.ascontiguousarray(np.asarray(inputs["q_w"], np.float32).T.reshape(
        NB, P, CI))
    kw = np.ascontiguousarray(np.asarray(inputs["k_w"], np.float32).T.reshape(
        NB, P, CI))
    vw = np.ascontiguousarray(
        (alpha * np.asarray(inputs["v_w"], np.float32)).T.reshape(NB, P, 512))
    qb = np.asarray(inputs["q_b"], np.float32).reshape(CI, 1)
    kb = np.asarray(inputs["k_b"], np.float32).reshape(CI, 1)
    vba = (alpha * np.asarray(inputs["v_b"], np.float32)).reshape(NB, P, 1)
    betat = np.full((P, 1), beta, np.float32)
    identr = np.eye(P, dtype=np.float32)
    import ml_dtypes
    identb = np.eye(P, dtype=ml_dtypes.bfloat16)

    zeros_qw = np.zeros_like(qw)
    zeros_vw = np.zeros_like(vw)
    zeros_b = np.zeros_like(qb)
    zeros_vba = np.zeros_like(vba)

    maps = []
    for core in range(8):
        b = core % 4
        xp = _pad_x(x[b])
        if core < 4:
            m = dict(xpad=xp, w1=w1s, b1=b1s, w2=w2s, b2=b2s,
                     qw=qw, kw=kw, vw=vw, qb=qb, kb=kb, vba=vba, betat=betat,
                     identr=identr, identb=identb)
        else:
            m = dict(xpad=xp, w1=w1c, b1=b1c, w2=w2c, b2=b2c,
                     qw=zeros_qw, kw=zeros_qw, vw=zeros_vw, qb=zeros_b,
                     kb=zeros_b, vba=zeros_vba, betat=betat,
                     identr=identr, identb=identb)
        maps.append(m)
    return maps


def kernel(**inputs):
    if "nc" not in _CACHE:
        _CACHE["nc"] = build()
    nc = _CACHE["nc"]
    maps = prep_inputs(inputs)
    res = run_bass_kernel_spmd(nc, maps, core_ids=list(range(8)))
    out = np.zeros((B, C, H, W), np.float32)
    for b in range(B):
        sa = res.results[b]["out"].reshape(C, H, W)
        ca = res.results[b + 4]["out"].reshape(C, H, W)
        out[b] = sa + ca
    return out



# revision 24
# speedup vs baseline: 1.0494x; 1.0494x over previous
"""DualAttention2d Trainium2 kernel.

Sharding: 8 cores = 4 samples x {spatial-attention branch, channel-attention
branch}. Core c < 4 computes the spatial branch of sample c; core c >= 4
computes the channel branch of sample c-4. Host sums the two branch outputs.

Single SPMD program; branch divergence via tc.If(partition_id < 4).

Layout notes:
- Feature maps on-chip as [4 blocks][128 chan, S] with S = 64*64 = 4096.
- Conv inputs live in a zero-padded [128, 66*66] buffer (1-px halo); a 3x3
  conv is 9 shifted matmuls accumulated in PSUM over 4 channel blocks.
- BN is folded into conv weights/bias on the host. alpha is folded into the
  v-projection, beta into the channel-attention softmax normalization.
- Matmuls run in float32r (full PE rate at N>=256, ~1e-4 rel precision).
  Attention probabilities are bf16; they are transposed for the o-matmul by
  PE transposes (128x128 tiles) evicted via ScalarE into two half-buffers.
- Conv1 is fused with the q/k/vT projections (st-pair outer loop, evict
  tiles consumed in SBUF); conv weights are SBUF-resident per output block.
- DMAs are batched (2-4 tiles per transfer) and split between the SP (HWDGE)
  and GpSimd (SWDGE) queues to keep issue cost off the critical path.
- The attention middle is software-pipelined over (group, blk) steps:
  logits matmuls run two steps ahead and softmax one step ahead of the PE
  transposes, so the in-order PE queue never waits on Act/DVE softmax
  latency. Transposes land 4-to-a-PSUM-tile and evict in one strided
  activation; the o-matmul accumulates all 4 channel blocks in a single
  pass (4 PSUM banks), reading each vT chunk once per group. The spatial
  branch skips c1t production entirely (only the channel branch reads it).
- The per-blk negmax is computed in stage L (right after the st loop, two
  steps before its consumer) so exp never waits at the tail of the DVE
  queue behind the lookahead logits copies.
- Cost model (TimelineSim): spatial branch ~1.19 ms, channel ~0.73 ms per
  core (vs ~1.38/0.73 ms before pipelining, ~1.75/0.73 ms phase-serial).
"""

import numpy as np

import concourse.bacc as bacc
import concourse.mybir as mybir
import concourse.tile as tile
from concourse.bass_utils import run_bass_kernel_spmd

B, C, H, W = 4, 512, 64, 64
S = H * W            # 4096
CI = 64              # q/k channels
P = 128
NB = C // P          # 4 channel blocks
PW = 66              # padded row width
PR = 66              # padded rows (1 zero row top/bottom)
PAD = PW * PR        # 4356
NST = S // 512       # 8 s-tiles of 512
NCH = S // P         # 32 s-chunks of 128
EPS = 1e-5

F32 = mybir.dt.float32
F32R = mybir.dt.float32r
BF16 = mybir.dt.bfloat16
AF = mybir.ActivationFunctionType
AX = mybir.AxisListType

_CACHE = {}


def _pad_view(xpad_ap, st, dy=1, dx=1):
    """View of padded buffer [128, PAD] covering s-tile `st` (8 image rows x 64
    cols) shifted by tap (dy, dx) in {0,1,2}^2. dy=dx=1 is the centered view."""
    v = xpad_ap.rearrange("p (r w) -> p r w", w=PW)
    r0 = st * 8 + dy
    return v[:, r0:r0 + 8, dx:dx + 64]


def build(branch=None):
    """branch=None: SPMD program with If/Else on partition id.
    branch="spatial"/"channel": single-branch program (analysis/timing)."""
    nc = bacc.Bacc("TRN2", target_bir_lowering=False, debug=False,
                   num_devices=8)

    # ---- I/O ----
    x_d = nc.dram_tensor("xpad", [NB, P, PAD], F32R, kind="ExternalInput")
    # conv weights pre-arranged host-side: [ob, tap, cb, ci, o]
    w1_d = nc.dram_tensor("w1", [NB, 36, P, P], F32R, kind="ExternalInput")
    b1_d = nc.dram_tensor("b1", [NB, P, 1], F32, kind="ExternalInput")
    w2_d = nc.dram_tensor("w2", [NB, 36, P, P], F32R, kind="ExternalInput")
    b2_d = nc.dram_tensor("b2", [NB, P, 1], F32, kind="ExternalInput")
    qw_d = nc.dram_tensor("qw", [NB, P, CI], F32R, kind="ExternalInput")
    kw_d = nc.dram_tensor("kw", [NB, P, CI], F32R, kind="ExternalInput")
    vw_d = nc.dram_tensor("vw", [NB, P, 512], F32R, kind="ExternalInput")
    qb_d = nc.dram_tensor("qb", [CI, 1], F32, kind="ExternalInput")
    kb_d = nc.dram_tensor("kb", [CI, 1], F32, kind="ExternalInput")
    vba_d = nc.dram_tensor("vba", [NB, P, 1], F32, kind="ExternalInput")
    beta_d = nc.dram_tensor("betat", [P, 1], F32, kind="ExternalInput")
    idr_d = nc.dram_tensor("identr", [P, P], F32R, kind="ExternalInput")
    idb_d = nc.dram_tensor("identb", [P, P], BF16, kind="ExternalInput")
    out_d = nc.dram_tensor("out", [NB, P, S], F32, kind="ExternalOutput")

    # ---- internal DRAM scratch ----
    s1_d = nc.dram_tensor("s1f", [NB, P, S], F32R, kind="Internal")
    c1t_d = nc.dram_tensor("c1t", [NCH, P, 512], F32R, kind="Internal")
    q_d = nc.dram_tensor("qs", [CI, S], F32R, kind="Internal")
    k_d = nc.dram_tensor("ks", [CI, S], F32R, kind="Internal")
    vt_d = nc.dram_tensor("vts", [NCH, P, 512], BF16, kind="Internal")

    with tile.TileContext(nc) as tc:
        from contextlib import ExitStack

        # ---- global pools (whole kernel) ----
        gctx = ExitStack()
        psA = gctx.enter_context(tc.tile_pool(name="psA", bufs=6,
                                              space="PSUM"))
        psT = gctx.enter_context(tc.tile_pool(name="psT", bufs=2,
                                              space="PSUM"))
        xpadp = gctx.enter_context(tc.tile_pool(name="xpadp", bufs=NB))
        consts = gctx.enter_context(tc.tile_pool(name="consts", bufs=1))
        b512 = gctx.enter_context(tc.tile_pool(name="b512", bufs=3))
        statp = gctx.enter_context(tc.tile_pool(name="statp", bufs=12))

        # ---- constants ----
        ident_r = consts.tile([P, P], F32R, name="ident_r")
        nc.sync.dma_start(ident_r[:], idr_d.ap())
        ident_b = consts.tile([P, P], BF16, name="ident_b")
        nc.sync.dma_start(ident_b[:], idb_d.ap())
        qw_t = [consts.tile([P, CI], F32R, name=f"qw{i}") for i in range(NB)]
        kw_t = [consts.tile([P, CI], F32R, name=f"kw{i}") for i in range(NB)]
        vw_t = [consts.tile([P, 512], F32R, name=f"vw{i}") for i in range(NB)]
        b1_t = [consts.tile([P, 1], F32, name=f"b1{i}") for i in range(NB)]
        b2_t = [consts.tile([P, 1], F32, name=f"b2{i}") for i in range(NB)]
        vba_t = [consts.tile([P, 1], F32, name=f"vba{i}") for i in range(NB)]
        qb_t = consts.tile([CI, 1], F32, name="qbt")
        kb_t = consts.tile([CI, 1], F32, name="kbt")
        beta_t = consts.tile([P, 1], F32, name="betat_sb")
        for i in range(NB):
            nc.sync.dma_start(qw_t[i][:], qw_d[i])
            nc.sync.dma_start(kw_t[i][:], kw_d[i])
            nc.sync.dma_start(vw_t[i][:], vw_d[i])
            nc.sync.dma_start(b1_t[i][:], b1_d[i])
            nc.sync.dma_start(b2_t[i][:], b2_d[i])
            nc.sync.dma_start(vba_t[i][:], vba_d[i])
        nc.sync.dma_start(qb_t[:], qb_d.ap())
        nc.sync.dma_start(kb_t[:], kb_d.ap())
        nc.sync.dma_start(beta_t[:], beta_d.ap())

        # ---- load padded input ----
        xpad = [xpadp.tile([P, PAD], F32R, tag="xp", name=f"xpad{i}")
                for i in range(NB)]
        for i in range(NB):
            hh = PAD // 2
            nc.sync.dma_start(xpad[i][:, :hh], x_d[i, :, :hh])
            nc.gpsimd.dma_start(xpad[i][:, hh:], x_d[i, :, hh:])

        def load_wres(wpool, w_dram, ob):
            """The 36 [128,128] stationaries of one conv output block."""
            wres = wpool.tile([P, 36 * P], F32R, tag="wres", name="wres")
            nc.sync.dma_start(
                wres[:].rearrange("p (k o) -> p k o", o=P),
                w_dram[ob].rearrange("k p o -> p k o"))
            return wres

        def conv1_pair(wres, ob, st0, bounce, b1ref):
            """One conv over s-tiles (st0, st0+1) for output block ob; returns
            the evicted [128,1024] relu tile; also writes s1_d and c1t_d."""
            ps = [psA.tile([P, 512], F32, tag="mm", name=f"c1p{sl}")
                  for sl in range(2)]
            for tci in range(36):
                cb, tap = tci // 9, tci % 9
                dy, dx = tap // 3, tap % 3
                for sl in range(2):
                    nc.tensor.matmul(
                        ps[sl][:], wres[:, tci * P:(tci + 1) * P],
                        _pad_view(xpad[cb][:], st0 + sl, dy, dx),
                        start=(tci == 0), stop=(tci == 35))
            sb = bounce.tile([P, 1024], F32R, tag="bn", name=f"sb{ob}")
            for sl in range(2):
                nc.scalar.activation(sb[:, sl * 512:(sl + 1) * 512],
                                     ps[sl][:], AF.Relu, bias=b1ref[ob][:])
            nc.gpsimd.dma_start(
                s1_d[ob, :, st0 * 512:(st0 + 2) * 512], sb[:])
            return sb

        def c1t_out(sb, ob, st0, tb4):
            """Transpose the pair-tile into c1t_d chunks (8 chunks)."""
            for sl in range(2):
                tb = tb4.tile([P, 512], F32R, tag="t4", name="tb")
                for j in range(4):
                    pt = psT.tile([P, P], F32R, tag="tp", name="pt")
                    nc.tensor.transpose(
                        pt[:], sb[:, sl * 512 + j * P:sl * 512 + (j + 1) * P],
                        ident_r[:])
                    nc.scalar.activation(tb[:, j * P:(j + 1) * P], pt[:],
                                         AF.Identity)
                st = st0 + sl
                nc.gpsimd.dma_start(
                    c1t_d.ap()[st * 4:st * 4 + 4, :, ob * P:(ob + 1) * P]
                    .rearrange("j p c -> p j c"),
                    tb[:].rearrange("p (j c) -> p j c", c=P))

        def spatial_middle():
            # long-lived attention inputs
            resctx = ExitStack()
            kqp = resctx.enter_context(tc.tile_pool(name="kqp", bufs=1))
            kg = kqp.tile([CI, S], F32R, tag="kg", name="kg")

            # ---- conv1 fused with q/k/vT production, st-pair outer ----
            # (no c1t_out here: c1t is only read by the channel branch)
            with ExitStack() as c1ctx:
                wp = c1ctx.enter_context(tc.tile_pool(name="wp1", bufs=2))
                bounce = c1ctx.enter_context(tc.tile_pool(name="bn1", bufs=5))
                vtbp = c1ctx.enter_context(tc.tile_pool(name="vtbp", bufs=2))
                for pair in range(NST // 2):
                    st0 = pair * 2
                    sbs = []
                    for ob in range(NB):
                        wres = load_wres(wp, w1_d.ap(), ob)
                        sb = conv1_pair(wres, ob, st0, bounce, b1_t)
                        sbs.append(sb)
                    # q, k, vT for the two s-tiles of this pair
                    for sl in range(2):
                        st = st0 + sl
                        ssl = slice(sl * 512, (sl + 1) * 512)
                        pq = psA.tile([CI, 512], F32, tag="mm", name="pq")
                        pk = psA.tile([CI, 512], F32, tag="mm", name="pk")
                        for cb in range(NB):
                            nc.tensor.matmul(pq[:], qw_t[cb][:],
                                             sbs[cb][:, ssl],
                                             start=(cb == 0),
                                             stop=(cb == NB - 1))
                        for cb in range(NB):
                            nc.tensor.matmul(pk[:], kw_t[cb][:],
                                             sbs[cb][:, ssl],
                                             start=(cb == 0),
                                             stop=(cb == NB - 1))
                        qsb = b512.tile([CI, 512], F32R, tag="bn",
                                        name="qsb")
                        nc.scalar.activation(qsb[:], pq[:], AF.Identity,
                                             bias=qb_t[:])
                        nc.gpsimd.dma_start(
                            q_d.ap()[:, st * 512:(st + 1) * 512], qsb[:])
                        nc.scalar.activation(kg[:, st * 512:(st + 1) * 512],
                                             pk[:], AF.Identity, bias=kb_t[:])
                        vtb = vtbp.tile([P, 2048], BF16, tag="vtb",
                                        name=f"vtb{st}")
                        for j in range(4):
                            pv = psA.tile([P, 512], F32, tag="mm", name="pv")
                            for cb in range(NB):
                                nc.tensor.matmul(
                                    pv[:],
                                    sbs[cb][:, sl * 512 + j * P:
                                            sl * 512 + (j + 1) * P],
                                    vw_t[cb][:], start=(cb == 0),
                                    stop=(cb == NB - 1))
                            nc.scalar.activation(
                                vtb[:, j * 512:(j + 1) * 512], pv[:],
                                AF.Identity)
                        nc.gpsimd.dma_start(
                            vt_d.ap()[st * 4:st * 4 + 4].rearrange(
                                "j p n -> p j n"),
                            vtb[:].rearrange("p (j n) -> p j n", n=512))

            # ---- attention, software-pipelined over (g, blk) steps ----
            # Stage L (logits matmuls + copies + maxes) runs two steps ahead
            # and stage SM (softmax: negmax/exp/recip/mul) one step ahead of
            # stage TR (PE transposes), so the in-order PE queue never waits
            # on Act/DVE softmax latency. The o-matmul accumulates all four
            # channel blocks in one pass (4 PSUM banks), reading each vT
            # chunk once per group.
            with ExitStack() as attctx:
                qgp = attctx.enter_context(tc.tile_pool(name="qgp", bufs=2))
                logp = attctx.enter_context(tc.tile_pool(name="logp", bufs=2))
                probp = attctx.enter_context(tc.tile_pool(name="probp",
                                                          bufs=2))
                attTpA = attctx.enter_context(tc.tile_pool(name="attTpA",
                                                           bufs=1))
                attTpB = attctx.enter_context(tc.tile_pool(name="attTpB",
                                                           bufs=1))
                vtip = attctx.enter_context(tc.tile_pool(name="vtip", bufs=3))
                s1rp = attctx.enter_context(tc.tile_pool(name="s1rp", bufs=1))
                NSTEP = NST * 4
                qgs, logst, smst, atts = {}, {}, {}, {}

                def issue_L(i):
                    g, blk = divmod(i, 4)
                    if blk == 0:
                        qg = qgp.tile([CI, 512], F32R, tag="qg", name="qg")
                        nc.sync.dma_start(
                            qg[:], q_d.ap()[:, g * 512:(g + 1) * 512])
                        qgs[g] = qg
                    qg = qgs[g]
                    logits = logp.tile([P, S], F32, tag="lg", name="logits")
                    pmax = statp.tile([P, 8], F32, tag="pm", name="pmax")
                    for st in range(NST):
                        pl = psA.tile([P, 512], F32, tag="mm", name="pl")
                        nc.tensor.matmul(
                            pl[:], qg[:, blk * P:(blk + 1) * P],
                            kg[:, st * 512:(st + 1) * 512],
                            start=True, stop=True)
                        nc.vector.tensor_copy(
                            logits[:, st * 512:(st + 1) * 512], pl[:])
                        nc.vector.reduce_max(pmax[:, st:st + 1], pl[:],
                                             axis=AX.X)
                    # negmax here (stage L, two steps early) rather than in
                    # SM: it only needs pmax, and hoisting it off the tail
                    # of the DVE queue gives exp a full step of slack
                    negmax = statp.tile([P, 1], F32, tag="st", name="negmax")
                    nc.vector.reduce_max(negmax[:], pmax[:], axis=AX.X,
                                         negate=True)
                    logst[i] = (logits, negmax)

                def issue_SM(i):
                    logits, negmax = logst.pop(i)
                    probs = probp.tile([P, S], BF16, tag="pb", name="probs")
                    rowsum = statp.tile([P, 1], F32, tag="st", name="rowsum")
                    nc.scalar.activation(probs[:], logits[:], AF.Exp,
                                         bias=negmax[:], accum_out=rowsum[:])
                    recip = statp.tile([P, 1], F32, tag="st", name="recip")
                    nc.vector.reciprocal(recip[:], rowsum[:])
                    nc.vector.tensor_scalar_mul(probs[:], probs[:], recip[:])
                    smst[i] = probs

                def issue_TR(i):
                    g, blk = divmod(i, 4)
                    if blk == 0:
                        attA = attTpA.tile([P, NCH * 256], BF16, tag="attA",
                                           name=f"attA{g}")
                        attB = attTpB.tile([P, NCH * 256], BF16, tag="attB",
                                           name=f"attB{g}")
                        atts[g] = (attA, attB)
                    attA, attB = atts[g]
                    probs = smst.pop(i)
                    for q4 in range(NCH // 4):
                        pt = psT.tile([P, 512], BF16, tag="tp", name="pt")
                        for k in range(4):
                            j = q4 * 4 + k
                            nc.tensor.transpose(
                                pt[:, k * P:(k + 1) * P],
                                probs[:, j * P:(j + 1) * P], ident_b[:])
                        ah = attA if q4 < NCH // 8 else attB
                        jh0 = (q4 * 4) % (NCH // 2)
                        ahv = ah[:].rearrange("p (j b c) -> p j b c",
                                              b=4, c=P)
                        nc.scalar.activation(
                            ahv[:, jh0:jh0 + 4, blk],
                            pt[:].rearrange("p (k c) -> p k c", c=P),
                            AF.Identity)

                def issue_PO(g):
                    attA, attB = atts.pop(g)
                    po = [psA.tile([P, 512], F32, tag="mm", name=f"po{i}")
                          for i in range(NB)]
                    for j4 in range(NCH // 4):
                        vt = vtip.tile([P, 4 * 512], BF16, tag="vti",
                                       name="vt")
                        nc.sync.dma_start(
                            vt[:].rearrange("p (j n) -> p j n", n=512),
                            vt_d.ap()[j4 * 4:j4 * 4 + 4].rearrange(
                                "j p n -> p j n"))
                        for jj in range(4):
                            j = j4 * 4 + jj
                            ahalf = attA if j < NCH // 2 else attB
                            jh = j % (NCH // 2)
                            for cb in range(NB):
                                nc.tensor.matmul(
                                    po[cb][:],
                                    vt[:, jj * 512 + cb * P:
                                       jj * 512 + (cb + 1) * P],
                                    ahalf[:, jh * 512:(jh + 1) * 512],
                                    start=(j == 0), stop=(j == NCH - 1))
                    s1r = s1rp.tile([P, NB * 512], F32R, tag="s1r",
                                    name="s1r")
                    nc.sync.dma_start(
                        s1r[:].rearrange("p (b n) -> p b n", n=512),
                        s1_d.ap()[:, :, g * 512:(g + 1) * 512].rearrange(
                            "b p n -> p b n"))
                    for cb in range(NB):
                        ob_sb = b512.tile([P, 512], F32, tag="bn",
                                          name="obsb")
                        nc.scalar.activation(ob_sb[:], po[cb][:],
                                             AF.Identity, bias=vba_t[cb][:])
                        nc.vector.tensor_add(
                            _pad_view(xpad[cb][:], g), ob_sb[:],
                            s1r[:, cb * 512:(cb + 1) * 512])

                issue_L(0)
                issue_L(1)
                issue_SM(0)
                for i in range(NSTEP):
                    if i + 2 < NSTEP:
                        issue_L(i + 2)
                    if i + 1 < NSTEP:
                        issue_SM(i + 1)
                    issue_TR(i)
                    if i % 4 == 3:
                        issue_PO(i // 4)
            resctx.close()
            conv2()

        def channel_middle():
            # ---- conv1 (st-pair outer) + c1T production ----
            with ExitStack() as c1ctx:
                wp = c1ctx.enter_context(tc.tile_pool(name="wp1c", bufs=2))
                bounce = c1ctx.enter_context(tc.tile_pool(name="bn1c",
                                                          bufs=3))
                tb4 = c1ctx.enter_context(tc.tile_pool(name="tb41c", bufs=2))
                for pair in range(NST // 2):
                    st0 = pair * 2
                    for ob in range(NB):
                        wres = load_wres(wp, w1_d.ap(), ob)
                        sb = conv1_pair(wres, ob, st0, bounce, b1_t)
                        c1t_out(sb, ob, st0, tb4)

            with ExitStack() as chctx:
                c1tp = chctx.enter_context(tc.tile_pool(name="c1tp", bufs=2))
                cattp = chctx.enter_context(tc.tile_pool(name="cattp",
                                                         bufs=NB))
                # G = c1 @ c1^T via transposed chunks
                pg = [psA.tile([P, 512], F32, tag="mm", name=f"pg{cb}")
                      for cb in range(NB)]
                for j2 in range(NCH // 2):
                    c1t = c1tp.tile([P, 1024], F32R, tag="c1t", name="c1tin")
                    nc.sync.dma_start(
                        c1t[:].rearrange("p (j n) -> p j n", n=512),
                        c1t_d.ap()[j2 * 2:j2 * 2 + 2].rearrange(
                            "j p n -> p j n"))
                    for jj in range(2):
                        j = j2 * 2 + jj
                        ch = c1t[:, jj * 512:(jj + 1) * 512]
                        for cb in range(NB):
                            nc.tensor.matmul(pg[cb][:],
                                             ch[:, cb * P:(cb + 1) * P],
                                             ch[:], start=(j == 0),
                                             stop=(j == NCH - 1))
                catt = []
                for cb in range(NB):
                    negmax = statp.tile([P, 1], F32, tag="st", name="negmax")
                    nc.vector.reduce_max(negmax[:], pg[cb][:], axis=AX.X,
                                         negate=True)
                    ct = cattp.tile([P, 512], F32R, tag="ct",
                                    name=f"catt{cb}")
                    rowsum = statp.tile([P, 1], F32, tag="st", name="rowsum")
                    nc.scalar.activation(ct[:], pg[cb][:], AF.Exp,
                                         bias=negmax[:], accum_out=rowsum[:])
                    recip = statp.tile([P, 1], F32, tag="st", name="recip")
                    nc.vector.reciprocal(recip[:], rowsum[:])
                    # fold beta in: catt = beta * softmax(G)
                    nc.vector.tensor_mul(recip[:], recip[:], beta_t[:])
                    nc.scalar.activation(ct[:], ct[:], AF.Identity,
                                         scale=recip[:])
                    catt.append(ct)
                for st in range(NST):
                    c1s = c1tp.tile([P, NB, 512], F32R, tag="c4", name="c1s")
                    nc.sync.dma_start(
                        c1s[:],
                        s1_d.ap()[:, :, st * 512:(st + 1) * 512].rearrange(
                            "b p n -> p b n"))
                    for kb in range(NB):
                        pc = psA.tile([P, 512], F32, tag="mm", name="pc")
                        for cb in range(NB):
                            nc.tensor.matmul(
                                pc[:], catt[cb][:, kb * P:(kb + 1) * P],
                                c1s[:, cb], start=(cb == 0),
                                stop=(cb == NB - 1))
                        nc.vector.tensor_add(
                            _pad_view(xpad[kb][:], st), pc[:], c1s[:, kb])
            conv2()

        def conv2():
            # st-outer so it can chase the middle's residual writes
            with ExitStack() as c2ctx:
                wp = c2ctx.enter_context(tc.tile_pool(name="wp2", bufs=2))
                bounce2 = c2ctx.enter_context(tc.tile_pool(name="bn2",
                                                           bufs=2))
                for pair in range(NST // 2):
                    st0 = pair * 2
                    for ob in range(NB):
                        wres = load_wres(wp, w2_d.ap(), ob)
                        ps = [psA.tile([P, 512], F32, tag="mm",
                                       name=f"c2p{sl}") for sl in range(2)]
                        for tci in range(36):
                            cb, tap = tci // 9, tci % 9
                            dy, dx = tap // 3, tap % 3
                            for sl in range(2):
                                nc.tensor.matmul(
                                    ps[sl][:], wres[:, tci * P:(tci + 1) * P],
                                    _pad_view(xpad[cb][:], st0 + sl, dy, dx),
                                    start=(tci == 0), stop=(tci == 35))
                        sb = bounce2.tile([P, 1024], F32, tag="bn",
                                          name=f"ob{ob}")
                        for sl in range(2):
                            nc.scalar.activation(
                                sb[:, sl * 512:(sl + 1) * 512], ps[sl][:],
                                AF.Relu, bias=b2_t[ob][:])
                        nc.gpsimd.dma_start(
                            out_d[ob, :, st0 * 512:(st0 + 2) * 512], sb[:])

        if branch == "spatial":
            spatial_middle()
        elif branch == "channel":
            channel_middle()
        else:
            pid = nc.partition_id()
            with tc.If(pid < 4) as cmp:
                spatial_middle()
            with cmp.Else():
                channel_middle()

        gctx.close()

    nc.compile()
    return nc


def _fold_conv(w, g, b, m, v):
    scale = g / np.sqrt(v + EPS)
    wf = (w * scale[:, None, None, None]).astype(np.float32)
    bf = (b - m * scale).astype(np.float32)
    # [O, CI, 3, 3] -> [ob, (cb tap), ci, o]
    wt = wf.transpose(2, 3, 1, 0).reshape(9, NB, P, NB, P).transpose(
        3, 1, 0, 2, 4).reshape(NB, 36, P, P)
    return np.ascontiguousarray(wt), bf.reshape(NB, P, 1)


def _pad_x(x):
    # x: [C, H, W] -> [NB, P, PAD]
    xp = np.zeros((NB, P, PR, PW), np.float32)
    xp[:, :, 1:65, 1:65] = x.reshape(NB, P, H, W)
    return xp.reshape(NB, P, PAD)


def prep_inputs(inputs):
    """Build the 8 per-core input maps from the full problem inputs."""
    x = np.asarray(inputs["x"], np.float32)
    alpha = float(np.asarray(inputs["alpha"]).reshape(-1)[0])
    beta = float(np.asarray(inputs["beta"]).reshape(-1)[0])

    w1s, b1s = _fold_conv(np.asarray(inputs["sa_w1"]), inputs["sa_g1"],
                          inputs["sa_b1"], inputs["sa_m1"], inputs["sa_v1"])
    w2s, b2s = _fold_conv(np.asarray(inputs["sa_w2"]), inputs["sa_g2"],
                          inputs["sa_b2"], inputs["sa_m2"], inputs["sa_v2"])
    w1c, b1c = _fold_conv(np.asarray(inputs["ca_w1"]), inputs["ca_g1"],
                          inputs["ca_b1"], inputs["ca_m1"], inputs["ca_v1"])
    w2c, b2c = _fold_conv(np.asarray(inputs["ca_w2"]), inputs["ca_g2"],
                          inputs["ca_b2"], inputs["ca_m2"], inputs["ca_v2"])

    qw = np.ascontiguousarray(np.asarray(inputs["q_w"], np.float32).T.reshape(
        NB, P, CI))
    kw = np.ascontiguousarray(np.asarray(inputs["k_w"], np.float32).T.reshape(
        NB, P, CI))
    vw = np.ascontiguousarray(
        (alpha * np.asarray(inputs["v_w"], np.float32)).T.reshape(NB, P, 512))
    qb = np.asarray(inputs["q_b"], np.float32).reshape(CI, 1)
    kb = np.asarray(inputs["k_b"], np.float32).reshape(CI, 1)
    vba = (alpha * np.asarray(inputs["v_b"], np.float32)).reshape(NB, P, 1)
    betat = np.full((P, 1), beta, np.float32)
    identr = np.eye(P, dtype=np.float32)
    import ml_dtypes
    identb = np.eye(P, dtype=ml_dtypes.bfloat16)

    zeros_qw = np.zeros_like(qw)
    zeros_vw = np.zeros_like(vw)
    zeros_b = np.zeros_like(qb)
    zeros_vba = np.zeros_like(vba)

    maps = []
    for core in range(8):
        b = core % 4
        xp = _pad_x(x[b])
        if core < 4:
            m = dict(xpad=xp, w1=w1s, b1=b1s, w2=w2s, b2=b2s,
                     qw=qw, kw=kw, vw=vw, qb=qb, kb=kb, vba=vba, betat=betat,
                     identr=identr, identb=identb)
        else:
            m = dict(xpad=xp, w1=w1c, b1=b1c, w2=w2c, b2=b2c,
                     qw=zeros_qw, kw=zeros_qw, vw=zeros_vw, qb=zeros_b,
                     kb=zeros_b, vba=zeros_vba, betat=betat,
                     identr=identr, identb=identb)
        maps.append(m)
    return maps


def kernel(**inputs):
    if "nc" not in _CACHE:
        _CACHE["nc"] = build()
    nc = _CACHE["nc"]
    maps = prep_inputs(inputs)
    res = run_bass_kernel_spmd(nc, maps, core_ids=list(range(8)))
    out = np.zeros((B, C, H, W), np.float32)
    for b in range(B):
        sa = res.results[b]["out"].reshape(C, H, W)
        ca = res.results[b + 4]["out"].reshape(C, H, W)
        out[b] = sa + ca
    return out



# revision 26
# speedup vs baseline: 1.2117x; 1.1546x over previous
"""DualAttention2d Trainium2 kernel.

Sharding: 8 cores = 4 samples x {spatial-attention branch, channel-attention
branch}. Core c < 4 computes the spatial branch of sample c; core c >= 4
computes the channel branch of sample c-4. Host sums the two branch outputs.

Single SPMD program; branch divergence via tc.If(partition_id < 4).

Layout notes:
- Feature maps on-chip as [4 blocks][128 chan, S] with S = 64*64 = 4096.
- Conv inputs live in a zero-padded [128, 66*66] buffer (1-px halo); a 3x3
  conv is 9 shifted matmuls accumulated in PSUM over 4 channel blocks.
- BN is folded into conv weights/bias on the host. alpha is folded into the
  v-projection, beta into the channel-attention softmax normalization.
- Matmuls run in float32r (full PE rate at N>=256, ~1e-4 rel precision).
  Attention probabilities are bf16; they are transposed for the o-matmul by
  PE transposes (128x128 tiles) evicted via ScalarE into two half-buffers.
- Conv1 is fused with the q/k/vT projections (st-pair outer loop, evict
  tiles consumed in SBUF); conv weights are SBUF-resident per output block.
- DMAs are batched (2-4 tiles per transfer) and split between the SP (HWDGE)
  and GpSimd (SWDGE) queues to keep issue cost off the critical path.
- The attention middle is software-pipelined over (group, blk) steps:
  logits matmuls run two steps ahead and softmax one step ahead of the PE
  transposes, so the in-order PE queue never waits on Act/DVE softmax
  latency. Transposes land 4-to-a-PSUM-tile and evict in one strided
  activation; the o-matmul accumulates all 4 channel blocks in a single
  pass (4 PSUM banks), reading each vT chunk once per group. The spatial
  branch skips c1t production entirely (only the channel branch reads it).
- The per-blk negmax is computed in stage L (right after the st loop, two
  steps before its consumer) so exp never waits at the tail of the DVE
  queue behind the lookahead logits copies.
- Cost model (TimelineSim): spatial branch ~1.19 ms, channel ~0.73 ms per
  core (vs ~1.38/0.73 ms before pipelining, ~1.75/0.73 ms phase-serial).
"""

import numpy as np

import concourse.bacc as bacc
import concourse.mybir as mybir
import concourse.tile as tile
from concourse.bass_utils import run_bass_kernel_spmd

B, C, H, W = 4, 512, 64, 64
S = H * W            # 4096
CI = 64              # q/k channels
P = 128
NB = C // P          # 4 channel blocks
PW = 66              # padded row width
PR = 66              # padded rows (1 zero row top/bottom)
PAD = PW * PR        # 4356
NST = S // 512       # 8 s-tiles of 512
NCH = S // P         # 32 s-chunks of 128
EPS = 1e-5

F32 = mybir.dt.float32
F32R = mybir.dt.float32r
BF16 = mybir.dt.bfloat16
AF = mybir.ActivationFunctionType
AX = mybir.AxisListType

_CACHE = {}


def _pad_view(xpad_ap, st, dy=1, dx=1):
    """View of padded buffer [128, PAD] covering s-tile `st` (8 image rows x 64
    cols) shifted by tap (dy, dx) in {0,1,2}^2. dy=dx=1 is the centered view."""
    v = xpad_ap.rearrange("p (r w) -> p r w", w=PW)
    r0 = st * 8 + dy
    return v[:, r0:r0 + 8, dx:dx + 64]


def build(branch=None):
    """branch=None: SPMD program with If/Else on partition id.
    branch="spatial"/"channel": single-branch program (analysis/timing)."""
    nc = bacc.Bacc("TRN2", target_bir_lowering=False, debug=False,
                   num_devices=8)

    # ---- I/O ----
    x_d = nc.dram_tensor("xpad", [NB, P, PAD], F32R, kind="ExternalInput")
    # conv weights pre-arranged host-side: [ob, tap, cb, ci, o]
    w1_d = nc.dram_tensor("w1", [NB, 36, P, P], F32R, kind="ExternalInput")
    b1_d = nc.dram_tensor("b1", [NB, P, 1], F32, kind="ExternalInput")
    w2_d = nc.dram_tensor("w2", [NB, 36, P, P], F32R, kind="ExternalInput")
    b2_d = nc.dram_tensor("b2", [NB, P, 1], F32, kind="ExternalInput")
    qw_d = nc.dram_tensor("qw", [NB, P, CI], F32R, kind="ExternalInput")
    kw_d = nc.dram_tensor("kw", [NB, P, CI], F32R, kind="ExternalInput")
    vw_d = nc.dram_tensor("vw", [NB, P, 512], F32R, kind="ExternalInput")
    qb_d = nc.dram_tensor("qb", [CI, 1], F32, kind="ExternalInput")
    kb_d = nc.dram_tensor("kb", [CI, 1], F32, kind="ExternalInput")
    vba_d = nc.dram_tensor("vba", [NB, P, 1], F32, kind="ExternalInput")
    beta_d = nc.dram_tensor("betat", [P, 1], F32, kind="ExternalInput")
    idr_d = nc.dram_tensor("identr", [P, P], F32R, kind="ExternalInput")
    idb_d = nc.dram_tensor("identb", [P, P], BF16, kind="ExternalInput")
    out_d = nc.dram_tensor("out", [NB, P, S], F32, kind="ExternalOutput")

    # ---- internal DRAM scratch ----
    s1_d = nc.dram_tensor("s1f", [NB, P, S], F32R, kind="Internal")
    c1t_d = nc.dram_tensor("c1t", [NCH, P, 512], F32R, kind="Internal")
    q_d = nc.dram_tensor("qs", [CI, S], F32R, kind="Internal")
    k_d = nc.dram_tensor("ks", [CI, S], F32R, kind="Internal")
    vt_d = nc.dram_tensor("vts", [NCH, P, 512], BF16, kind="Internal")

    with tile.TileContext(nc) as tc:
        from contextlib import ExitStack

        # ---- global pools (whole kernel) ----
        gctx = ExitStack()
        psA = gctx.enter_context(tc.tile_pool(name="psA", bufs=6,
                                              space="PSUM"))
        psT = gctx.enter_context(tc.tile_pool(name="psT", bufs=2,
                                              space="PSUM"))
        xpadp = gctx.enter_context(tc.tile_pool(name="xpadp", bufs=NB))
        consts = gctx.enter_context(tc.tile_pool(name="consts", bufs=1))
        b512 = gctx.enter_context(tc.tile_pool(name="b512", bufs=3))
        statp = gctx.enter_context(tc.tile_pool(name="statp", bufs=12))

        # ---- constants ----
        ident_r = consts.tile([P, P], F32R, name="ident_r")
        nc.sync.dma_start(ident_r[:], idr_d.ap())
        ident_b = consts.tile([P, P], BF16, name="ident_b")
        nc.sync.dma_start(ident_b[:], idb_d.ap())
        qw_t = [consts.tile([P, CI], F32R, name=f"qw{i}") for i in range(NB)]
        kw_t = [consts.tile([P, CI], F32R, name=f"kw{i}") for i in range(NB)]
        vw_t = [consts.tile([P, 512], F32R, name=f"vw{i}") for i in range(NB)]
        b1_t = [consts.tile([P, 1], F32, name=f"b1{i}") for i in range(NB)]
        b2_t = [consts.tile([P, 1], F32, name=f"b2{i}") for i in range(NB)]
        vba_t = [consts.tile([P, 1], F32, name=f"vba{i}") for i in range(NB)]
        qb_t = consts.tile([CI, 1], F32, name="qbt")
        kb_t = consts.tile([CI, 1], F32, name="kbt")
        beta_t = consts.tile([P, 1], F32, name="betat_sb")
        for i in range(NB):
            nc.sync.dma_start(qw_t[i][:], qw_d[i])
            nc.sync.dma_start(kw_t[i][:], kw_d[i])
            nc.sync.dma_start(vw_t[i][:], vw_d[i])
            nc.sync.dma_start(b1_t[i][:], b1_d[i])
            nc.sync.dma_start(b2_t[i][:], b2_d[i])
            nc.sync.dma_start(vba_t[i][:], vba_d[i])
        nc.sync.dma_start(qb_t[:], qb_d.ap())
        nc.sync.dma_start(kb_t[:], kb_d.ap())
        nc.sync.dma_start(beta_t[:], beta_d.ap())

        # ---- load padded input ----
        xpad = [xpadp.tile([P, PAD], F32R, tag="xp", name=f"xpad{i}")
                for i in range(NB)]
        for i in range(NB):
            hh = PAD // 2
            nc.sync.dma_start(xpad[i][:, :hh], x_d[i, :, :hh])
            nc.gpsimd.dma_start(xpad[i][:, hh:], x_d[i, :, hh:])

        def load_wres(wpool, w_dram, ob):
            """The 36 [128,128] stationaries of one conv output block."""
            wres = wpool.tile([P, 36 * P], F32R, tag="wres", name="wres")
            nc.sync.dma_start(
                wres[:].rearrange("p (k o) -> p k o", o=P),
                w_dram[ob].rearrange("k p o -> p k o"))
            return wres

        def conv1_pair(wres, ob, st0, bounce, b1ref):
            """One conv over s-tiles (st0, st0+1) for output block ob; returns
            the evicted [128,1024] relu tile; also writes s1_d and c1t_d."""
            ps = [psA.tile([P, 512], F32, tag="mm", name=f"c1p{sl}")
                  for sl in range(2)]
            for tci in range(36):
                cb, tap = tci // 9, tci % 9
                dy, dx = tap // 3, tap % 3
                for sl in range(2):
                    nc.tensor.matmul(
                        ps[sl][:], wres[:, tci * P:(tci + 1) * P],
                        _pad_view(xpad[cb][:], st0 + sl, dy, dx),
                        start=(tci == 0), stop=(tci == 35))
            sb = bounce.tile([P, 1024], F32R, tag="bn", name=f"sb{ob}")
            for sl in range(2):
                nc.scalar.activation(sb[:, sl * 512:(sl + 1) * 512],
                                     ps[sl][:], AF.Relu, bias=b1ref[ob][:])
            nc.gpsimd.dma_start(
                s1_d[ob, :, st0 * 512:(st0 + 2) * 512], sb[:])
            return sb

        def c1t_out(sb, ob, st0, tb4):
            """Transpose the pair-tile into c1t_d chunks (8 chunks)."""
            for sl in range(2):
                tb = tb4.tile([P, 512], F32R, tag="t4", name="tb")
                for j in range(4):
                    pt = psT.tile([P, P], F32R, tag="tp", name="pt")
                    nc.tensor.transpose(
                        pt[:], sb[:, sl * 512 + j * P:sl * 512 + (j + 1) * P],
                        ident_r[:])
                    nc.scalar.activation(tb[:, j * P:(j + 1) * P], pt[:],
                                         AF.Identity)
                st = st0 + sl
                nc.gpsimd.dma_start(
                    c1t_d.ap()[st * 4:st * 4 + 4, :, ob * P:(ob + 1) * P]
                    .rearrange("j p c -> p j c"),
                    tb[:].rearrange("p (j c) -> p j c", c=P))

        def spatial_middle():
            # long-lived attention inputs
            resctx = ExitStack()
            kqp = resctx.enter_context(tc.tile_pool(name="kqp", bufs=1))
            kg = kqp.tile([CI, S], F32R, tag="kg", name="kg")

            # ---- conv1 fused with q/k/vT production, st-pair outer ----
            # (no c1t_out here: c1t is only read by the channel branch)
            with ExitStack() as c1ctx:
                wp = c1ctx.enter_context(tc.tile_pool(name="wp1", bufs=2))
                bounce = c1ctx.enter_context(tc.tile_pool(name="bn1", bufs=5))
                vtbp = c1ctx.enter_context(tc.tile_pool(name="vtbp", bufs=2))
                for pair in range(NST // 2):
                    st0 = pair * 2
                    sbs = []
                    for ob in range(NB):
                        wres = load_wres(wp, w1_d.ap(), ob)
                        sb = conv1_pair(wres, ob, st0, bounce, b1_t)
                        sbs.append(sb)
                    # q, k, vT for the two s-tiles of this pair
                    for sl in range(2):
                        st = st0 + sl
                        ssl = slice(sl * 512, (sl + 1) * 512)
                        pq = psA.tile([CI, 512], F32, tag="mm", name="pq")
                        pk = psA.tile([CI, 512], F32, tag="mm", name="pk")
                        for cb in range(NB):
                            nc.tensor.matmul(pq[:], qw_t[cb][:],
                                             sbs[cb][:, ssl],
                                             start=(cb == 0),
                                             stop=(cb == NB - 1))
                        for cb in range(NB):
                            nc.tensor.matmul(pk[:], kw_t[cb][:],
                                             sbs[cb][:, ssl],
                                             start=(cb == 0),
                                             stop=(cb == NB - 1))
                        qsb = b512.tile([CI, 512], F32R, tag="bn",
                                        name="qsb")
                        nc.scalar.activation(qsb[:], pq[:], AF.Identity,
                                             bias=qb_t[:])
                        nc.gpsimd.dma_start(
                            q_d.ap()[:, st * 512:(st + 1) * 512], qsb[:])
                        nc.scalar.activation(kg[:, st * 512:(st + 1) * 512],
                                             pk[:], AF.Identity, bias=kb_t[:])
                        vtb = vtbp.tile([P, 2048], BF16, tag="vtb",
                                        name=f"vtb{st}")
                        for j in range(4):
                            pv = psA.tile([P, 512], F32, tag="mm", name="pv")
                            for cb in range(NB):
                                nc.tensor.matmul(
                                    pv[:],
                                    sbs[cb][:, sl * 512 + j * P:
                                            sl * 512 + (j + 1) * P],
                                    vw_t[cb][:], start=(cb == 0),
                                    stop=(cb == NB - 1))
                            nc.scalar.activation(
                                vtb[:, j * 512:(j + 1) * 512], pv[:],
                                AF.Identity)
                        nc.gpsimd.dma_start(
                            vt_d.ap()[st * 4:st * 4 + 4].rearrange(
                                "j p n -> p j n"),
                            vtb[:].rearrange("p (j n) -> p j n", n=512))

            # ---- attention, software-pipelined over (g, blk) steps ----
            # Stage L (logits matmuls + copies + maxes) runs two steps ahead
            # and stage SM (softmax: negmax/exp/recip/mul) one step ahead of
            # stage TR (PE transposes), so the in-order PE queue never waits
            # on Act/DVE softmax latency. The o-matmul accumulates all four
            # channel blocks in one pass (4 PSUM banks), reading each vT
            # chunk once per group.
            with ExitStack() as attctx:
                qgp = attctx.enter_context(tc.tile_pool(name="qgp", bufs=2))
                logp = attctx.enter_context(tc.tile_pool(name="logp", bufs=2))
                probp = attctx.enter_context(tc.tile_pool(name="probp",
                                                          bufs=2))
                attTpA = attctx.enter_context(tc.tile_pool(name="attTpA",
                                                           bufs=1))
                attTpB = attctx.enter_context(tc.tile_pool(name="attTpB",
                                                           bufs=1))
                vtip = attctx.enter_context(tc.tile_pool(name="vtip", bufs=3))
                s1rp = attctx.enter_context(tc.tile_pool(name="s1rp", bufs=1))
                NSTEP = NST * 4
                qgs, logst, smst, atts = {}, {}, {}, {}

                def issue_L(i):
                    g, blk = divmod(i, 4)
                    if blk == 0:
                        qg = qgp.tile([CI, 512], F32R, tag="qg", name="qg")
                        nc.sync.dma_start(
                            qg[:], q_d.ap()[:, g * 512:(g + 1) * 512])
                        qgs[g] = qg
                    qg = qgs[g]
                    logits = logp.tile([P, S], F32, tag="lg", name="logits")
                    pmax = statp.tile([P, 8], F32, tag="pm", name="pmax")
                    for st in range(NST):
                        pl = psA.tile([P, 512], F32, tag="mm", name="pl")
                        nc.tensor.matmul(
                            pl[:], qg[:, blk * P:(blk + 1) * P],
                            kg[:, st * 512:(st + 1) * 512],
                            start=True, stop=True)
                        nc.vector.tensor_copy(
                            logits[:, st * 512:(st + 1) * 512], pl[:])
                        nc.vector.reduce_max(pmax[:, st:st + 1], pl[:],
                                             axis=AX.X)
                    # negmax here (stage L, two steps early) rather than in
                    # SM: it only needs pmax, and hoisting it off the tail
                    # of the DVE queue gives exp a full step of slack
                    negmax = statp.tile([P, 1], F32, tag="st", name="negmax")
                    nc.vector.reduce_max(negmax[:], pmax[:], axis=AX.X,
                                         negate=True)
                    logst[i] = (logits, negmax)

                def issue_SM(i):
                    logits, negmax = logst.pop(i)
                    probs = probp.tile([P, S], BF16, tag="pb", name="probs")
                    rowsum = statp.tile([P, 1], F32, tag="st", name="rowsum")
                    nc.scalar.activation(probs[:], logits[:], AF.Exp,
                                         bias=negmax[:], accum_out=rowsum[:])
                    recip = statp.tile([P, 1], F32, tag="st", name="recip")
                    nc.vector.reciprocal(recip[:], rowsum[:])
                    nc.vector.tensor_scalar_mul(probs[:], probs[:], recip[:])
                    smst[i] = probs

                def issue_TR(i):
                    g, blk = divmod(i, 4)
                    if blk == 0:
                        attA = attTpA.tile([P, NCH * 256], BF16, tag="attA",
                                           name=f"attA{g}")
                        attB = attTpB.tile([P, NCH * 256], BF16, tag="attB",
                                           name=f"attB{g}")
                        atts[g] = (attA, attB)
                    attA, attB = atts[g]
                    probs = smst.pop(i)
                    for q4 in range(NCH // 4):
                        pt = psT.tile([P, 512], BF16, tag="tp", name="pt")
                        for k in range(4):
                            j = q4 * 4 + k
                            nc.tensor.transpose(
                                pt[:, k * P:(k + 1) * P],
                                probs[:, j * P:(j + 1) * P], ident_b[:])
                        ah = attA if q4 < NCH // 8 else attB
                        jh0 = (q4 * 4) % (NCH // 2)
                        ahv = ah[:].rearrange("p (j b c) -> p j b c",
                                              b=4, c=P)
                        nc.scalar.activation(
                            ahv[:, jh0:jh0 + 4, blk],
                            pt[:].rearrange("p (k c) -> p k c", c=P),
                            AF.Identity)

                def issue_PO(g):
                    attA, attB = atts.pop(g)
                    po = [psA.tile([P, 512], F32, tag="mm", name=f"po{i}")
                          for i in range(NB)]
                    for j4 in range(NCH // 4):
                        vt = vtip.tile([P, 4 * 512], BF16, tag="vti",
                                       name="vt")
                        nc.sync.dma_start(
                            vt[:].rearrange("p (j n) -> p j n", n=512),
                            vt_d.ap()[j4 * 4:j4 * 4 + 4].rearrange(
                                "j p n -> p j n"))
                        for jj in range(4):
                            j = j4 * 4 + jj
                            ahalf = attA if j < NCH // 2 else attB
                            jh = j % (NCH // 2)
                            for cb in range(NB):
                                nc.tensor.matmul(
                                    po[cb][:],
                                    vt[:, jj * 512 + cb * P:
                                       jj * 512 + (cb + 1) * P],
                                    ahalf[:, jh * 512:(jh + 1) * 512],
                                    start=(j == 0), stop=(j == NCH - 1))
                    s1r = s1rp.tile([P, NB * 512], F32R, tag="s1r",
                                    name="s1r")
                    nc.sync.dma_start(
                        s1r[:].rearrange("p (b n) -> p b n", n=512),
                        s1_d.ap()[:, :, g * 512:(g + 1) * 512].rearrange(
                            "b p n -> p b n"))
                    for cb in range(NB):
                        ob_sb = b512.tile([P, 512], F32, tag="bn",
                                          name="obsb")
                        nc.scalar.activation(ob_sb[:], po[cb][:],
                                             AF.Identity, bias=vba_t[cb][:])
                        nc.vector.tensor_add(
                            _pad_view(xpad[cb][:], g), ob_sb[:],
                            s1r[:, cb * 512:(cb + 1) * 512])

                issue_L(0)
                issue_L(1)
                issue_SM(0)
                for i in range(NSTEP):
                    if i + 2 < NSTEP:
                        issue_L(i + 2)
                    if i + 1 < NSTEP:
                        issue_SM(i + 1)
                    issue_TR(i)
                    if i % 4 == 3:
                        issue_PO(i // 4)
            resctx.close()
            conv2()

        def channel_middle():
            # ---- conv1 (st-pair outer) + c1T production ----
            with ExitStack() as c1ctx:
                wp = c1ctx.enter_context(tc.tile_pool(name="wp1c", bufs=2))
                bounce = c1ctx.enter_context(tc.tile_pool(name="bn1c",
                                                          bufs=3))
                tb4 = c1ctx.enter_context(tc.tile_pool(name="tb41c", bufs=2))
                for pair in range(NST // 2):
                    st0 = pair * 2
                    for ob in range(NB):
                        wres = load_wres(wp, w1_d.ap(), ob)
                        sb = conv1_pair(wres, ob, st0, bounce, b1_t)
                        c1t_out(sb, ob, st0, tb4)

            with ExitStack() as chctx:
                c1tp = chctx.enter_context(tc.tile_pool(name="c1tp", bufs=2))
                cattp = chctx.enter_context(tc.tile_pool(name="cattp",
                                                         bufs=NB))
                # G = c1 @ c1^T via transposed chunks
                pg = [psA.tile([P, 512], F32, tag="mm", name=f"pg{cb}")
                      for cb in range(NB)]
                for j2 in range(NCH // 2):
                    c1t = c1tp.tile([P, 1024], F32R, tag="c1t", name="c1tin")
                    nc.sync.dma_start(
                        c1t[:].rearrange("p (j n) -> p j n", n=512),
                        c1t_d.ap()[j2 * 2:j2 * 2 + 2].rearrange(
                            "j p n -> p j n"))
                    for jj in range(2):
                        j = j2 * 2 + jj
                        ch = c1t[:, jj * 512:(jj + 1) * 512]
                        for cb in range(NB):
                            nc.tensor.matmul(pg[cb][:],
                                             ch[:, cb * P:(cb + 1) * P],
                                             ch[:], start=(j == 0),
                                             stop=(j == NCH - 1))
                catt = []
                for cb in range(NB):
                    negmax = statp.tile([P, 1], F32, tag="st", name="negmax")
                    nc.vector.reduce_max(negmax[:], pg[cb][:], axis=AX.X,
                                         negate=True)
                    ct = cattp.tile([P, 512], F32R, tag="ct",
                                    name=f"catt{cb}")
                    rowsum = statp.tile([P, 1], F32, tag="st", name="rowsum")
                    nc.scalar.activation(ct[:], pg[cb][:], AF.Exp,
                                         bias=negmax[:], accum_out=rowsum[:])
                    recip = statp.tile([P, 1], F32, tag="st", name="recip")
                    nc.vector.reciprocal(recip[:], rowsum[:])
                    # fold beta in: catt = beta * softmax(G)
                    nc.vector.tensor_mul(recip[:], recip[:], beta_t[:])
                    nc.scalar.activation(ct[:], ct[:], AF.Identity,
                                         scale=recip[:])
                    catt.append(ct)
                for st in range(NST):
                    c1s = c1tp.tile([P, NB, 512], F32R, tag="c4", name="c1s")
                    nc.sync.dma_start(
                        c1s[:],
                        s1_d.ap()[:, :, st * 512:(st + 1) * 512].rearrange(
                            "b p n -> p b n"))
                    for kb in range(NB):
                        pc = psA.tile([P, 512], F32, tag="mm", name="pc")
                        for cb in range(NB):
                            nc.tensor.matmul(
                                pc[:], catt[cb][:, kb * P:(kb + 1) * P],
                                c1s[:, cb], start=(cb == 0),
                                stop=(cb == NB - 1))
                        nc.vector.tensor_add(
                            _pad_view(xpad[kb][:], st), pc[:], c1s[:, kb])
            conv2()

        def conv2():
            # st-outer so it can chase the middle's residual writes
            with ExitStack() as c2ctx:
                wp = c2ctx.enter_context(tc.tile_pool(name="wp2", bufs=2))
                bounce2 = c2ctx.enter_context(tc.tile_pool(name="bn2",
                                                           bufs=2))
                for pair in range(NST // 2):
                    st0 = pair * 2
                    for ob in range(NB):
                        wres = load_wres(wp, w2_d.ap(), ob)
                        ps = [psA.tile([P, 512], F32, tag="mm",
                                       name=f"c2p{sl}") for sl in range(2)]
                        for tci in range(36):
                            cb, tap = tci // 9, tci % 9
                            dy, dx = tap // 3, tap % 3
                            for sl in range(2):
                                nc.tensor.matmul(
                                    ps[sl][:], wres[:, tci * P:(tci + 1) * P],
                                    _pad_view(xpad[cb][:], st0 + sl, dy, dx),
                                    start=(tci == 0), stop=(tci == 35))
                        sb = bounce2.tile([P, 1024], F32, tag="bn",
                                          name=f"ob{ob}")
                        for sl in range(2):
                            nc.scalar.activation(
                                sb[:, sl * 512:(sl + 1) * 512], ps[sl][:],
                                AF.Relu, bias=b2_t[ob][:])
                        nc.gpsimd.dma_start(
                            out_d[ob, :, st0 * 512:(st0 + 2) * 512], sb[:])

        if branch == "spatial":
            spatial_middle()
        elif branch == "channel":
            channel_middle()
        else:
            pid = nc.partition_id()
            with tc.If(pid < 4) as cmp:
                spatial_middle()
            with cmp.Else():
                channel_middle()

        gctx.close()

    nc.compile()
    return nc


def _fold_conv(w, g, b, m, v):
    scale = g / np.sqrt(v + EPS)
    wf = (w * scale[:, None, None, None]).astype(np.float32)
    bf = (b - m * scale).astype(np.float32)
    # [O, CI, 3, 3] -> [ob, (cb tap), ci, o]
    wt = wf.transpose(2, 3, 1, 0).reshape(9, NB, P, NB, P).transpose(
        3, 1, 0, 2, 4).reshape(NB, 36, P, P)
    return np.ascontiguousarray(wt), bf.reshape(NB, P, 1)


def _pad_x(x):
    # x: [C, H, W] -> [NB, P, PAD]
    xp = np.zeros((NB, P, PR, PW), np.float32)
    xp[:, :, 1:65, 1:65] = x.reshape(NB, P, H, W)
    return xp.reshape(NB, P, PAD)


def prep_inputs(inputs):
    """Build the 8 per-core input maps from the full problem inputs."""
    x = np.asarray(inputs["x"], np.float32)
    alpha = float(np.asarray(inputs["alpha"]).reshape(-1)[0])
    beta = float(np.asarray(inputs["beta"]).reshape(-1)[0])

    w1s, b1s = _fold_conv(np.asarray(inputs["sa_w1"]), inputs["sa_g1"],
                          inputs["sa_b1"], inputs["sa_m1"], inputs["sa_v1"])
    w2s, b2s = _fold_conv(np.asarray(inputs["sa_w2"]), inputs["sa_g2"],
                          inputs["sa_b2"], inputs["sa_m2"], inputs["sa_v2"])
    w1c, b1c = _fold_conv(np.asarray(inputs["ca_w1"]), inputs["ca_g1"],
                          inputs["ca_b1"], inputs["ca_m1"], inputs["ca_v1"])
    w2c, b2c = _fold_conv(np.asarray(inputs["ca_w2"]), inputs["ca_g2"],
                          inputs["ca_b2"], inputs["ca_m2"], inputs["ca_v2"])

    qw = np.ascontiguousarray(np.asarray(inputs["q_w"], np.float32).T.reshape(
        NB, P, CI))
    kw = np.ascontiguousarray(np.asarray(inputs["k_w"], np.float32).T.reshape(
        NB, P, CI))
    vw = np.ascontiguousarray(
        (alpha * np.asarray(inputs["v_w"], np.float32)).T.reshape(NB, P, 512))
    qb = np.asarray(inputs["q_b"], np.float32).reshape(CI, 1)
    kb = np.asarray(inputs["k_b"], np.float32).reshape(CI, 1)
    vba = (alpha * np.asarray(inputs["v_b"], np.float32)).reshape(NB, P, 1)
    betat = np.full((P, 1), beta, np.float32)
    identr = np.eye(P, dtype=np.float32)
    import ml_dtypes
    identb = np.eye(P, dtype=ml_dtypes.bfloat16)

    zeros_qw = np.zeros_like(qw)
    zeros_vw = np.zeros_like(vw)
    zeros_b = np.zeros_like(qb)
    zeros_vba = np.zeros_like(vba)

    maps = []
    for core in range(8):
        b = core % 4
        xp = _pad_x(x[b])
        if core < 4:
            m = dict(xpad=xp, w1=w1s, b1=b1s, w2=w2s, b2=b2s,
                     qw=qw, kw=kw, vw=vw, qb=qb, kb=kb, vba=vba, betat=betat,
                     identr=identr, identb=identb)
        else:
            m = dict(xpad=xp, w1=w1c, b1=b1c, w2=w2c, b2=b2c,
                     qw=zeros_qw, kw=zeros_qw, vw=zeros_vw, qb=zeros_b,
                     kb=zeros_b, vba=zeros_vba, betat=betat,
                     identr=identr, identb=identb)
        maps.append(m)
    return maps


def kernel(**inputs):
    if "nc" not in _CACHE:
        _CACHE["nc"] = build()
    nc = _CACHE["nc"]
    maps = prep_inputs(inputs)
    res = run_bass_kernel_spmd(nc, maps, core_ids=list(range(8)))
    out = np.zeros((B, C, H, W), np.float32)
    for b in range(B):
        sa = res.results[b]["out"].reshape(C, H, W)
        ca = res.results[b + 4]["out"].reshape(C, H, W)
        out[b] = sa + ca
    return out

